# revision 13
# baseline (speedup 1.0000x reference)
"""CausalBoW (causal mean pooling) Trainium2 Bass kernel.

y[b, t, :] = mean(x[b, 0:t+1, :]) = cumsum(x, axis=1) / (t+1)

Full input x: [8, 4096, 1024] f32. Sharded batch-parallel: one batch of
[4096, 1024] per NeuronCore (8 cores).

Numerics: the harness gate is rel_err < 2e-2 (normalized); plain bf16 for
both the streamed input and the stored output keeps the end-to-end error
at ~3e-3 under max-normalized and ~2.5e-3 under L2-normalized metrics
while halving HBM traffic vs fp32 (8 MiB in + 8 MiB out per core). All
matmul weights are exactly 0/1; accumulation is fp32 in PSUM, so the only
error sources are the bf16 input/output quantization and the bf16
tile-sum table.

Per-core algorithm: blocked scan, T on partitions in 32 tiles of 128 rows,
processed in chunks (CHS) software-pipelined at tile granularity so the
compute/output of chunk g overlaps the input stream of chunk g+1:
  per chunk g (cb tiles at tile offset off):
    per tile i in chunk: DMA x tile in;
      PSUM S[i-off, :] += colsum(x_i) (one-hot selector MM)
    evict S into s2 rows [off, off+cb) as bf16
  per tile i (one chunk behind the stream), per 512-half, with a one-tile
  lag between the carry-free and carry-dependent matmuls so the in-order
  PE queue always holds independent work:
    z  = tri.T @ x_i                 local inclusive cumsum (group start)
    z += ones[0:i].T @ s2[0:i]       carry (prefix of tile sums, stop)
    y_i = z * (1/(t+1))    per-partition scale on PSUM->SBUF evict (bf16)
    DMA y tile out.
Tile 0 needs no carry, so its cumsum + evict + store are issued inside
chunk 0's stream — the output stream starts ~at the first tile's arrival.

The PE p-state ramps to full clock only with sustained execution, so a
short warm-up burst of throwaway matmuls on a zeroed scratch tile runs
while the first input tile is still in flight.

All 0/1 weight tables and the 1/(t+1) table are passed as extra DRAM
inputs (written before launch) rather than NEFF-inline tensors, and their
SBUF loads are triggered from the scalar queue: the sync queue's head
belongs to the input stream, which starts several microseconds earlier
as a result.

Engine roles: PE matmuls; ACT h0-evicts + s2 assembly + its hop DMA
triggers; DVE h1-evicts + warmup memset; sync HWDGE issues the input
stream and the drain-phase stores; gpsimd SWDGE issues the steady-state
output stores.
"""

import sys

for _p in ("/opt/trn_rl_repo",):
    if _p not in sys.path:
        sys.path.insert(0, _p)

import ml_dtypes
import numpy as np

import concourse.bass as bass
import concourse.mybir as mybir
import concourse.tile as tile
from concourse import bacc
from concourse.bass_utils import run_bass_kernel_spmd

B, T, C = 8, 4096, 1024
P = 128            # partition tile rows
NT = T // P        # 32 row-tiles
HALF = 512         # PSUM bank free-dim for f32
NH = C // HALF     # 2 halves
NWARM = 8          # PE warm-up matmuls
CHS = [8, 8, 8, 4, 4]          # chunk sizes (tiles); small final chunks
assert sum(CHS) == NT          # shorten the drain after the input stream
COFF = [sum(CHS[:b]) for b in range(len(CHS))]   # chunk tile offsets

F32 = mybir.dt.float32
BF16 = mybir.dt.bfloat16


def _consts():
    # lhsT for local inclusive cumsum: out = lhsT.T @ rhs, want
    # out[t, c] = sum_{s<=t} x[s, c] => lhsT[s, t] = 1 iff s <= t.
    tri_np = np.triu(np.ones((P, P), dtype=ml_dtypes.bfloat16))
    # all-ones weights: carry for tile i is ones[0:i].T @ s2[0:i]
    ones_np = np.ones((NT, P), dtype=ml_dtypes.bfloat16)
    # banded one-hot-column selector for routing colsum(x_i) into PSUM row
    # j: bnd[:, (7-j) : (7-j+cb)] has ones exactly in column j.
    bnd_np = np.zeros((P, 15), dtype=ml_dtypes.bfloat16)
    bnd_np[:, 7] = 1.0
    # inv[p, i] = 1 / (i*128 + p + 1)
    inv_np = (
        1.0 / np.arange(1, T + 1, dtype=np.float64)
    ).astype(np.float32).reshape(NT, P).T.copy()
    return tri_np, ones_np, bnd_np, inv_np


def _build_nc() -> bass.Bass:
    nc = bacc.Bacc(trn_type="TRN2")

    xh = nc.declare_dram_parameter("xh", [T, C], BF16, isOutput=False)
    tri_d = nc.declare_dram_parameter("tri", [P, P], BF16, isOutput=False)
    ones_d = nc.declare_dram_parameter("ones", [NT, P], BF16, isOutput=False)
    bnd_d = nc.declare_dram_parameter("bnd", [P, 15], BF16, isOutput=False)
    inv_d = nc.declare_dram_parameter("inv", [P, NT], F32, isOutput=False)
    y = nc.declare_dram_parameter("y", [T, C], BF16, isOutput=True)

    with tile.TileContext(nc) as tc:
        with (
            tc.tile_pool(name="consts", bufs=1) as cpool,
            tc.tile_pool(name="xpool", bufs=NT) as xpool,
            tc.tile_pool(name="ypool", bufs=8) as ypool,
            tc.tile_pool(name="s2p", bufs=1) as s2p,
            tc.tile_pool(name="stmp", bufs=2) as stmp,
            tc.tile_pool(name="ps_s", bufs=4, space="PSUM") as ps_s,
            tc.tile_pool(name="ps_z", bufs=4, space="PSUM") as ps_z,
        ):
            # const loads on the scalar queue: sync's queue head stays
            # reserved for the input stream
            bnd_sb = cpool.tile([P, 15], BF16)
            nc.scalar.dma_start(bnd_sb[:], bnd_d.ap())
            tri_sb = cpool.tile([P, P], BF16)
            nc.scalar.dma_start(tri_sb[:], tri_d.ap())
            inv_sb = cpool.tile([P, NT], F32)
            nc.scalar.dma_start(inv_sb[:], inv_d.ap())
            ones_sb = cpool.tile([NT, P], BF16)
            nc.scalar.dma_start(ones_sb[:], ones_d.ap())

            s2_sb = s2p.tile([NT, C], BF16)

            xhs = [None] * NT

            def load_and_colsum(g: int, j: int, s_ps):
                """DMA tile j of chunk g in, accumulate its column sums."""
                i = COFF[g] + j
                cb = CHS[g]
                # Each DMA engine runs at ~1/16 of HBM bandwidth; split the
                # first tile across queue slots so the pipeline starts
                # promptly. (All splits stay on sync: the scalar queue's
                # head carries the const loads.)
                nsplit = 4 if i < 1 else 1
                ps = P // nsplit
                xht = xpool.tile([P, C], BF16, name=f"xht{i}", tag="x")
                for s in range(nsplit):
                    rs = slice(s * ps, (s + 1) * ps)
                    gs = slice(i * P + s * ps, i * P + (s + 1) * ps)
                    nc.sync.dma_start(xht[rs, :], xh.ap()[gs, :])
                xhs[i] = xht
                lhs_j = bnd_sb[:, 7 - j : 7 - j + cb]
                for h in range(NH):
                    hs = slice(h * HALF, (h + 1) * HALF)
                    nc.tensor.matmul(
                        s_ps[h][:], lhsT=lhs_j, rhs=xht[:, hs],
                        start=(j == 0), stop=(j == cb - 1),
                    )

            def assemble_s2(g: int, s_ps):
                """Evict chunk-g tile-sums into s2 rows as bf16.

                DVE/ACT writes must start at partition 0/32/64/96, so evict
                to base-0 temporaries and DMA (any partition) into s2 rows.
                """
                cb = CHS[g]
                r0 = COFF[g]
                th = stmp.tile([cb, C], BF16, name=f"th{g}", tag="th")
                for h in range(NH):
                    hs = slice(h * HALF, (h + 1) * HALF)
                    nc.scalar.copy(th[:, hs], s_ps[h][:])
                # scalar HWDGE: the th copy runs on ACT, so its hop trigger
                # follows in the same queue with no cross-engine sem
                nc.scalar.dma_start(s2_sb[r0 : r0 + cb, :], th[:])

            zps = [None] * NT

            def phase_c_tri(i: int):
                """Local-cumsum matmuls for tile i (no carry dependency)."""
                zps[i] = []
                for h in range(NH):
                    zp = ps_z.tile([P, HALF], F32, name=f"zp{i}_{h}", tag="z")
                    zps[i].append(zp)
                    hs = slice(h * HALF, (h + 1) * HALF)
                    nc.tensor.matmul(
                        zp[:], lhsT=tri_sb[:], rhs=xhs[i][:, hs],
                        start=True, stop=(i == 0),
                    )

            def phase_c_fin(i: int):
                """Carry matmul + scale-evict + store for tile i."""
                yt = ypool.tile([P, C], BF16, name=f"yt{i}", tag="y")
                if i > 0:
                    for h in range(NH):
                        hs = slice(h * HALF, (h + 1) * HALF)
                        nc.tensor.matmul(
                            zps[i][h][:],
                            lhsT=ones_sb[0:i, :],
                            rhs=s2_sb[0:i, hs],
                            start=False, stop=True,
                        )
                # evict with per-partition 1/(t+1) scale; halves split
                # across ACT and DVE
                for h in range(NH):
                    hs = slice(h * HALF, (h + 1) * HALF)
                    if h == 0:
                        nc.scalar.mul(yt[:, hs], zps[i][h][:],
                                      inv_sb[:, i : i + 1])
                    else:
                        nc.vector.tensor_scalar_mul(
                            yt[:, hs], zps[i][h][:], inv_sb[:, i : i + 1]
                        )
                # drain phase: the input stream is done and sync idle — its
                # HWDGE triggers are ~1.6x cheaper than gpsimd's SWDGE
                dma_eng = nc.sync if i >= 24 else nc.gpsimd
                nsplit = 2 if i >= NT - 2 else 1
                ps = P // nsplit
                for s in range(nsplit):
                    rs = slice(s * ps, (s + 1) * ps)
                    gs = slice(i * P + s * ps, i * P + (s + 1) * ps)
                    dma_eng.dma_start(y.ap()[gs, :], yt[rs, :])

            # Software pipeline: interleave chunk g's input stream + column
            # sums with chunk g-1's phase-C at tile granularity, with a
            # one-tile lag between each tile's tri and carry matmuls so the
            # in-order PE queue always holds independent work while s2
            # assembles.
            LAG = 1
            pending: list = []

            def emit_tri(i: int):
                phase_c_tri(i)
                pending.append(i)
                if len(pending) > LAG:
                    phase_c_fin(pending.pop(0))

            tri_cursor = 1  # tile 0's phase-C is issued inside chunk 0
            for g in range(len(CHS)):
                s_ps = [
                    ps_s.tile([CHS[g], HALF], F32, name=f"sps{g}_{h}",
                              tag="s")
                    for h in range(NH)
                ]
                lim = COFF[g]  # phase-C may cover all tiles of prior chunks
                start = tri_cursor
                for j in range(CHS[g]):
                    target = start + (lim - start) * (j + 1) // CHS[g]
                    while tri_cursor < target:
                        emit_tri(tri_cursor)
                        tri_cursor += 1
                    load_and_colsum(g, j, s_ps)
                    if g == 0 and j == 0:
                        # tile 0 has no carry: run its cumsum + evict +
                        # store immediately so the output stream starts
                        # with the input stream
                        phase_c_tri(0)
                        phase_c_fin(0)
                assemble_s2(g, s_ps)
            while tri_cursor < NT:
                emit_tri(tri_cursor)
                tri_cursor += 1
            while pending:
                phase_c_fin(pending.pop(0))

    nc.compile()
    return nc


_NC_CACHE: list = []


def _get_nc() -> bass.Bass:
    if not _NC_CACHE:
        _NC_CACHE.append(_build_nc())
    return _NC_CACHE[0]


def _run(x: np.ndarray, **kwargs):
    x = np.ascontiguousarray(np.asarray(x), dtype=np.float32)
    assert x.shape == (B, T, C), x.shape
    nc = _get_nc()
    xh = x.astype(ml_dtypes.bfloat16)
    tri_np, ones_np, bnd_np, inv_np = _consts()
    in_maps = [
        {
            "xh": xh[b],
            "tri": tri_np,
            "ones": ones_np,
            "bnd": bnd_np,
            "inv": inv_np,
        }
        for b in range(B)
    ]
    return run_bass_kernel_spmd(nc, in_maps, core_ids=list(range(B)), **kwargs)


def kernel(x: np.ndarray) -> np.ndarray:
    res = _run(x)
    return np.stack(
        [r["y"].astype(np.float32) for r in res.results], axis=0
    )
